# revision 21
# baseline (speedup 1.0000x reference)
"""Trainium2 Bass kernel for nn_MHAttentionLayer_64587718197528.

Reference computation (B=4, L=1024, D_MODEL=1024, S=2048, T=NUM_TOKENS=1000,
H=16, E=256, D_LLM=4096):
    q = (X @ Wq.T + bq)            [B*L, H*E]      X = target_embedding
    k = (SE @ Wk.T + bk)           [S, H*E]        SE = source_embedding
    v = (VE @ Wv.T + bv)           [S, H*E]        VE = value_embedding
    scores[b,h,l,s] = q . k / 16 ; A = softmax_s ; out = A @ v
    y = out @ Wo.T + bo            [B*L, D_LLM]

Sharding: tensor-parallel over heads. Core i owns heads {2i, 2i+1} (an
e-slice of 512 of the H*E dim). Each core computes its q/k/v projections,
attention for its 2 heads, and a partial out-projection
  partial_i = attn_out_i @ Wo[:, sl_i].T          [B*L, D_LLM]
The host sums the 8 partials and adds bo (linearity of the projection).

All matmul operands are bf16 (1.0 PE cycles/row, same as fp32r, but half
the DMA/SBUF traffic); PSUM accumulation and the output partial are fp32.
Phases (PE is kept continuously busy; rel err ~1e-3 vs 2e-2 gate):
  Qproj: all 8 l-chunks of qT projected up front (xt read ONCE), covering
         the st/vt/weight DMA window that previously starved the PE.
  KV:    kT[512,2048] = Wk_i @ SE.T and v[2048,512] = VE_aug @ Wv_aug
         (bias for v folded via ones-row augmentation), SBUF-resident;
         st/vt staged whole-row [128,2048] for 4KB DMA lines.
  Attn+Proj interleaved per l-chunk: scoresT in PSUM, exp on ACT
         (scale=1/16, no max subtraction -- |scaled scores| < ~8),
         softmax denominators via DVE accumulation + ones-matmul
         partition-reduce, outT normalized by reciprocal broadcast; then
         the out-projection for that l-chunk immediately (spreads the
         67MB fp32 partial write across the whole attention span).
"""
import numpy as np

# ---- problem constants (hardcoded per contract) ----
B, L, D = 4, 1024, 1024
S, T = 2048, 1000
H, E = 16, 256
DL = 4096
BL = B * L            # 4096 query rows
EC = 512              # e-slice per core (2 heads)
NCORES = 8
T1 = T + 1            # augmented contraction for v bias

_CACHE = {}


def _build_nc():
    from contextlib import ExitStack

    import concourse.tile as tile
    from concourse import bacc, mybir

    F32 = mybir.dt.float32
    F32R = mybir.dt.float32r
    BF16 = mybir.dt.bfloat16
    AF = mybir.ActivationFunctionType
    MUL = mybir.AluOpType.mult
    ADD = mybir.AluOpType.add

    nc = bacc.Bacc("TRN2", target_bir_lowering=False, debug=False,
                   num_devices=NCORES)

    xt = nc.dram_tensor("xt", [D, BL], BF16, kind="ExternalInput")
    st = nc.dram_tensor("st", [T, S], BF16, kind="ExternalInput")
    vt = nc.dram_tensor("vt", [T1, S], BF16, kind="ExternalInput")
    wqt = nc.dram_tensor("wqt", [D, EC], BF16, kind="ExternalInput")
    wkt = nc.dram_tensor("wkt", [T, EC], BF16, kind="ExternalInput")
    wvt = nc.dram_tensor("wvt", [T1, EC], BF16, kind="ExternalInput")
    wot = nc.dram_tensor("wot", [EC, DL], BF16, kind="ExternalInput")
    bq_d = nc.dram_tensor("bq", [EC], F32, kind="ExternalInput")
    bk_d = nc.dram_tensor("bk", [EC], F32, kind="ExternalInput")
    out_d = nc.dram_tensor("out", [BL, DL], F32, kind="ExternalOutput")

    NKD = 8            # k-tiles for D=1024
    NKT = 8            # k-tiles for T=1000/T1=1001 (last partial)
    NLC = BL // 512    # 8 l-chunks
    NST = S // 128     # 16 s-tiles

    def kp_of(kk, total):
        return min(128, total - kk * 128)

    with tile.TileContext(nc) as tc:
        with ExitStack() as root:
            root.enter_context(
                nc.allow_low_precision(reason="bf16 matmul pipeline"))

            # ---- persistent pools ----
            consts = root.enter_context(tc.tile_pool(name="consts", bufs=1))
            kvp = root.enter_context(tc.tile_pool(name="kv", bufs=1))
            qtp = root.enter_context(tc.tile_pool(name="qt", bufs=1))
            wqp = root.enter_context(tc.tile_pool(name="wq", bufs=1))
            wop = root.enter_context(tc.tile_pool(name="wo", bufs=1))

            ones_f = consts.tile([128, 128], F32, name="ones_f")
            nc.vector.memset(ones_f[:], 1.0)
            ones_m = consts.tile([128, 128], F32R, name="ones_m")
            nc.vector.tensor_copy(ones_m[:], ones_f[:])
            bqbk_t = consts.tile([128, 8], F32, name="bqbk_t")
            bq_t = bqbk_t[:, 0:4]
            bk_t = bqbk_t[:, 4:8]

            # kT: 4 e-tiles x [128, S]; v: 4 groups [128, 4*EC]
            kt_sb = [kvp.tile([128, S], BF16, name=f"kt{m}", tag=f"kt{m}")
                     for m in range(4)]
            v_sb = [kvp.tile([128, 4 * EC], BF16, name=f"v{g}", tag=f"v{g}")
                    for g in range(4)]
            # qT for ALL l-chunks: 4 e-tiles x [128, BL]
            qt_all = [qtp.tile([128, BL], BF16, name=f"qT{m}", tag=f"qT{m}")
                      for m in range(4)]
            # Wo k-tiles: 4 x [128, DL], fully resident before proj
            wo_sb = [wop.tile([128, DL], BF16, name=f"wo{ke}", tag=f"wo{ke}")
                     for ke in range(4)]

            # ---- phase Qproj (covers weight DMA) + KV ----
            with ExitStack() as ph:
                ph.enter_context(nc.named_scope("qkv"))
                wkv_pool = ph.enter_context(tc.tile_pool(name="wkv", bufs=1))

                wq_t, wk_t, wv_t = [], [], []
                for kk in range(NKD):
                    wq_t.append(wqp.tile([128, EC], BF16, name=f"wq{kk}"))
                for kk in range(NKT):
                    wk_t.append(wkv_pool.tile([128, EC], BF16, name=f"wk{kk}"))
                    wv_t.append(wkv_pool.tile([128, EC], BF16, name=f"wv{kk}"))

                # qproj: for each 512-wide l-chunk, 4 PSUM tiles (one per
                # e-tile), double-buffered generations so evictions overlap
                # the next chunk's matmuls. Evictions rotate over
                # ACT/DVE/Pool. First DMAs are interleaved wq/xq so the
                # first matmul's operands land as early as possible.
                with ExitStack() as qph:
                    xq_pool = qph.enter_context(
                        tc.tile_pool(name="xq", bufs=20))
                    psq = qph.enter_context(
                        tc.tile_pool(name="psq", bufs=2, space="PSUM"))
                    for lcq in range(NLC):
                        ps_q = [psq.tile([128, 512], F32, tag=f"psq{m}",
                                         name=f"psq{m}") for m in range(4)]
                        for kk in range(NKD):
                            if lcq == 0:
                                # first weight tile races the first xq on a
                                # separate queue; the rest ride scalar
                                weng = nc.sync if kk == 0 else nc.scalar
                                weng.dma_start(
                                    wq_t[kk][:],
                                    wqt[kk * 128:(kk + 1) * 128, :])
                            xq = xq_pool.tile([128, 512], BF16, tag="xq",
                                              name="xq")
                            # SP queue carries only the xq stream (in-order
                            # queue: anything else ahead of an xq tile
                            # delays it and starves the PE)
                            nc.sync.dma_start(
                                xq[:], xt[kk * 128:(kk + 1) * 128,
                                          lcq * 512:(lcq + 1) * 512])
                            if lcq == 0 and kk == 2:
                                # biases off the critical first-DMA path but
                                # in time for the first eviction
                                nc.scalar.dma_start(
                                    bqbk_t[:, 0:4],
                                    bq_d.ap().rearrange("(m p) -> p m",
                                                        p=128))
                                nc.scalar.dma_start(
                                    bqbk_t[:, 4:8],
                                    bk_d.ap().rearrange("(m p) -> p m",
                                                        p=128))
                            for m in range(4):
                                nc.tensor.matmul(
                                    ps_q[m][:],
                                    wq_t[kk][:, m * 128:(m + 1) * 128],
                                    xq[:], start=(kk == 0),
                                    stop=(kk == NKD - 1))
                        for m in range(4):
                            dst = qt_all[m][:, lcq * 512:(lcq + 1) * 512]
                            if m == 0:
                                nc.scalar.activation(
                                    dst, ps_q[m][:], AF.Identity,
                                    bias=bq_t[:, m:m + 1])
                            elif m == 1:
                                nc.vector.tensor_scalar_add(
                                    dst, ps_q[m][:], bq_t[:, m:m + 1])
                            elif m == 2:
                                nc.vector.tensor_scalar_add(
                                    dst, ps_q[m][:], bq_t[:, m:m + 1])
                            else:
                                nc.scalar.activation(
                                    dst, ps_q[m][:], AF.Identity,
                                    bias=bq_t[:, m:m + 1])
                        # stream wk/wv weight tiles on the scalar queue
                        # (keeps the SP queue clear for xq)
                        if lcq % 2 == 1:
                            kk = lcq // 2
                            kp = kp_of(kk, T)
                            kp1 = kp_of(kk, T1)
                            nc.scalar.dma_start(
                                wk_t[kk][:kp, :],
                                wkt[kk * 128:kk * 128 + kp, :])
                            nc.scalar.dma_start(
                                wv_t[kk][:kp1, :],
                                wvt[kk * 128:kk * 128 + kp1, :])
                    for kk in range(4, NKT):
                        kp = kp_of(kk, T)
                        kp1 = kp_of(kk, T1)
                        nc.scalar.dma_start(
                            wk_t[kk][:kp, :], wkt[kk * 128:kk * 128 + kp, :])
                        nc.scalar.dma_start(
                            wv_t[kk][:kp1, :],
                            wvt[kk * 128:kk * 128 + kp1, :])

                # ---- KV: kT = Wk_i @ SE.T ; v = VE_aug @ Wv_aug ----
                # st/vt stream per (sc, kk); wo preload spread across sc.
                sk_pool = ph.enter_context(tc.tile_pool(name="sk", bufs=12))
                sv_pool = ph.enter_context(tc.tile_pool(name="sv", bufs=12))
                psk = ph.enter_context(
                    tc.tile_pool(name="psk", bufs=1, space="PSUM"))
                psv = ph.enter_context(
                    tc.tile_pool(name="psv", bufs=1, space="PSUM"))
                for sc in range(S // 512):
                    nc.scalar.dma_start(
                        wo_sb[sc][:], wot[sc * 128:(sc + 1) * 128, :])
                    ps_k = [psk.tile([128, 512], F32, tag=f"psk{m}",
                                     name=f"psk{m}") for m in range(4)]
                    for kk in range(NKT):
                        kp = kp_of(kk, T)
                        stt = sk_pool.tile([128, 512], BF16, tag="stt",
                                           name="stt")
                        nc.sync.dma_start(
                            stt[:kp, :], st[kk * 128:kk * 128 + kp,
                                            sc * 512:(sc + 1) * 512])
                        for m in range(4):
                            nc.tensor.matmul(
                                ps_k[m][:],
                                wk_t[kk][:kp, m * 128:(m + 1) * 128],
                                stt[:kp, :],
                                start=(kk == 0), stop=(kk == NKT - 1))
                    for m in range(4):
                        nc.scalar.activation(
                            kt_sb[m][:, sc * 512:(sc + 1) * 512], ps_k[m][:],
                            AF.Identity, bias=bk_t[:, m:m + 1])
                    ps_v = [psv.tile([128, 512], F32, tag=f"psv{j}",
                                     name=f"psv{j}") for j in range(4)]
                    for kk in range(NKT):
                        kp1 = kp_of(kk, T1)
                        vtt = sv_pool.tile([128, 512], BF16, tag="vtt",
                                           name="vtt")
                        nc.sync.dma_start(
                            vtt[:kp1, :], vt[kk * 128:kk * 128 + kp1,
                                             sc * 512:(sc + 1) * 512])
                        for j in range(4):
                            nc.tensor.matmul(
                                ps_v[j][:],
                                vtt[:kp1, j * 128:(j + 1) * 128],
                                wv_t[kk][:kp1, :],
                                start=(kk == 0), stop=(kk == NKT - 1))
                    for j in range(4):
                        nc.scalar.activation(
                            v_sb[sc][:, j * EC:(j + 1) * EC], ps_v[j][:],
                            AF.Copy)

            # ---- fused attention + out-projection per l-chunk ----
            with ExitStack() as ph:
                ph.enter_context(nc.named_scope("attnproj"))
                a_pool = ph.enter_context(tc.tile_pool(name="ap", bufs=1))
                acc_pool = ph.enter_context(tc.tile_pool(name="accp", bufs=2))
                bc_pool = ph.enter_context(tc.tile_pool(name="bcp", bufs=2))
                o_pool = ph.enter_context(tc.tile_pool(name="op", bufs=2))
                pev_pool = ph.enter_context(tc.tile_pool(name="pev", bufs=6))
                ps_sT_p = ph.enter_context(
                    tc.tile_pool(name="ps_sT", bufs=2, space="PSUM"))
                ps_o_p = ph.enter_context(
                    tc.tile_pool(name="ps_o", bufs=1, space="PSUM"))
                psp_p = ph.enter_context(
                    tc.tile_pool(name="psp", bufs=2, space="PSUM"))

                # Deferred softmax-denominator finish for the previous head:
                # the ones-matmul (partition reduce + broadcast), reciprocal
                # and o-normalization are emitted only after the NEXT head's
                # first scores, so the PE never waits on the DVE chain.
                pend = {"v": None}

                def flush_den():
                    p = pend["v"]
                    if p is None:
                        return
                    acc_d, ps_os_p, o0, o1 = p
                    ps_b = psp_p.tile([128, 1024], F32, tag="pp",
                                      name="pp")[:, 0:512]
                    nc.tensor.matmul(ps_b, ones_m[:], acc_d[:],
                                     start=True, stop=True)
                    bc = bc_pool.tile([128, 512], F32, tag="bc", name="bc")
                    nc.vector.reciprocal_approx_fast(out=bc[:], in_=ps_b)
                    nc.vector.tensor_tensor(o0[:], ps_os_p[0][:], bc[:], MUL)
                    nc.vector.tensor_tensor(o1[:], ps_os_p[1][:], bc[:], MUL)
                    pend["v"] = None

                def attn_head(lc, h, a_t, o_t):
                    # scores per 128-s-tile in alternating 1-bank PSUM tiles
                    # (bufs=2) so the next scores never wait on the exp
                    # draining the previous tile. AV matmuls lag 4 s-tiles
                    # behind; exps/accumulation have a ~3us window.
                    acc_d = acc_pool.tile([128, 512], F32R, tag="accd",
                                          name="accd")
                    acc_p = acc_pool.tile([128, 512], F32R, tag="accp",
                                          name="accp")
                    ps_os = [ps_o_p.tile([128, 512], F32, tag=f"os{et}",
                                         name=f"os{et}") for et in range(2)]

                    def av_stt(stt):
                        for et in range(2):
                            nc.tensor.matmul(
                                ps_os[et][:],
                                v_sb[stt // 4][:, (stt % 4) * EC + h * E
                                               + et * 128:
                                               (stt % 4) * EC + h * E
                                               + (et + 1) * 128],
                                a_t[stt // 4][:, (stt % 4) * 512:
                                              (stt % 4 + 1) * 512],
                                start=(stt == 0), stop=(stt == NST - 1))

                    AVLAG = 4
                    for stt in range(NST):
                        ps = ps_sT_p.tile([128, 512], F32, tag="sT",
                                          name="sT")
                        for et in range(2):
                            m = 2 * h + et
                            nc.tensor.matmul(
                                ps[:],
                                kt_sb[m][:, stt * 128:(stt + 1) * 128],
                                qt_all[m][:, lc * 512:(lc + 1) * 512],
                                start=(et == 0), stop=(et == 1))
                        if stt == 0:
                            flush_den()
                        a_ap = a_t[stt // 4][:, (stt % 4) * 512:
                                             (stt % 4 + 1) * 512]
                        nc.scalar.activation(a_ap, ps[:], AF.Exp,
                                             scale=0.0625)
                        # denominator partial sums: DVE (even) / Pool (odd)
                        if stt % 2 == 0:
                            if stt == 0:
                                nc.vector.tensor_copy(acc_d[:], a_ap)
                            else:
                                nc.vector.tensor_tensor(acc_d[:], acc_d[:],
                                                        a_ap, ADD)
                        else:
                            if stt == 1:
                                nc.gpsimd.tensor_copy(acc_p[:], a_ap)
                            else:
                                nc.gpsimd.tensor_tensor(acc_p[:], acc_p[:],
                                                        a_ap, ADD)
                        if stt >= AVLAG:
                            av_stt(stt - AVLAG)
                    for stt in range(NST - AVLAG, NST):
                        av_stt(stt)
                    nc.vector.tensor_tensor(acc_d[:], acc_d[:], acc_p[:],
                                            ADD)
                    pend["v"] = (acc_d, ps_os, o_t[2 * h], o_t[2 * h + 1])

                def proj_lc(lc, o_t):
                    # partial[lt,dc] = sum_ke o_t[ke].T @ wo ; dch pairs share
                    # the ke stationary tile (halved LDWEIGHTS traffic).
                    # Groups run in pairs with the ke contraction split in
                    # halves (ke 0-1 of both groups first): h1's o tiles are
                    # only needed ~1.9us into the pair, hiding the normalize
                    # latency of the head just flushed.
                    flush_den()
                    nout = 0
                    for lt in range(4):
                        for dpp in range(2):
                            pps = [psp_p.tile([128, 1024], F32, tag="pp",
                                              name="pp") for _ in range(2)]
                            for kep in range(2):
                                for ke in (2 * kep, 2 * kep + 1):
                                    for g in range(2):
                                        for dch in range(2):
                                            dc = (2 * dpp + g) * 2 + dch
                                            nc.tensor.matmul(
                                                pps[g][:, dch * 512:
                                                       (dch + 1) * 512],
                                                o_t[ke][:, lt * 128:
                                                        (lt + 1) * 128],
                                                wo_sb[ke][:, dc * 512:
                                                          (dc + 1) * 512],
                                                start=(ke == 0),
                                                stop=(ke == 3))
                            for g in range(2):
                                ev = pev_pool.tile([128, 1024], F32,
                                                   tag="pev", name="pev")
                                nc.vector.tensor_copy(ev[:, 0:512],
                                                      pps[g][:, 0:512])
                                nc.scalar.activation(ev[:, 512:1024],
                                                     pps[g][:, 512:1024],
                                                     AF.Copy)
                                # alternate output queues to halve the
                                # in-order drain at the kernel tail
                                oeng = nc.sync if nout % 2 == 0 else nc.scalar
                                nout += 1
                                oeng.dma_start(
                                    out_d[(lc * 4 + lt) * 128:
                                          (lc * 4 + lt + 1) * 128,
                                          (2 * dpp + g) * 1024:
                                          (2 * dpp + g + 1) * 1024], ev[:])

                for lc in range(NLC):
                    a_t = [a_pool.tile([128, 4 * 512], BF16, tag=f"a{g}",
                                       name=f"a{g}") for g in range(4)]
                    o_t = [o_pool.tile([128, 512], BF16, tag=f"o{m}",
                                       name=f"o{m}") for m in range(4)]
                    attn_head(lc, 0, a_t, o_t)
                    attn_head(lc, 1, a_t, o_t)
                    proj_lc(lc, o_t)

    nc.compile()
    return nc


def _get_nc():
    if "nc" not in _CACHE:
        _CACHE["nc"] = _build_nc()
    return _CACHE["nc"]


def _build_in_maps(inputs):
    return _prep(**{k: inputs[k] for k in (
        "target_embedding", "source_embedding", "value_embedding",
        "Wq", "bq", "Wk", "bk", "Wv", "bv", "Wo")})


def _prep(target_embedding, source_embedding, value_embedding,
          Wq, bq, Wk, bk, Wv, bv, Wo):
    import ml_dtypes
    mmd = ml_dtypes.bfloat16
    f32 = np.float32
    X = np.asarray(target_embedding, f32).reshape(BL, D)
    xt = np.ascontiguousarray(X.T)                       # [D, BL]
    st = np.ascontiguousarray(np.asarray(source_embedding, f32).T)  # [T, S]
    vt_base = np.asarray(value_embedding, f32).T         # [T, S]
    vt = np.ascontiguousarray(
        np.concatenate([vt_base, np.ones((1, S), f32)], axis=0))  # [T1, S]
    WqT = np.asarray(Wq, f32).T                          # [D, H*E]
    WkT = np.asarray(Wk, f32).T                          # [T, H*E]
    WvT = np.asarray(Wv, f32).T                          # [T, H*E]
    WoT = np.asarray(Wo, f32).T                          # [H*E, DL]
    bq = np.asarray(bq, f32)
    bk = np.asarray(bk, f32)
    bv = np.asarray(bv, f32)

    xt_c = xt.astype(mmd)
    st_c = st.astype(mmd)
    vt_c = vt.astype(mmd)
    in_maps = []
    for i in range(NCORES):
        sl = slice(i * EC, (i + 1) * EC)
        wvt_i = np.ascontiguousarray(
            np.concatenate([WvT[:, sl], bv[sl][None, :]], axis=0))  # [T1, EC]
        in_maps.append({
            "xt": xt_c,
            "st": st_c,
            "vt": vt_c,
            "wqt": np.ascontiguousarray(WqT[:, sl]).astype(mmd),
            "wkt": np.ascontiguousarray(WkT[:, sl]).astype(mmd),
            "wvt": wvt_i.astype(mmd),
            "wot": np.ascontiguousarray(WoT[sl, :]).astype(mmd),
            "bq": np.ascontiguousarray(bq[sl]),
            "bk": np.ascontiguousarray(bk[sl]),
        })
    return in_maps


def kernel(target_embedding, source_embedding, value_embedding,
           Wq, bq, Wk, bk, Wv, bv, Wo, bo):
    from concourse.bass_utils import run_bass_kernel_spmd

    in_maps = _prep(target_embedding, source_embedding, value_embedding,
                    Wq, bq, Wk, bk, Wv, bv, Wo)
    _CACHE["in_maps"] = in_maps
    nc = _get_nc()
    res = run_bass_kernel_spmd(nc, in_maps, list(range(NCORES)))

    acc = res.results[0]["out"].astype(np.float64)
    for i in range(1, NCORES):
        acc += res.results[i]["out"]
    out = (acc + np.asarray(bo, np.float64)[None, :]).astype(np.float32)
    return out.reshape(B, L, DL)


# revision 22
# speedup vs baseline: 1.0259x; 1.0259x over previous
"""Trainium2 Bass kernel for nn_MHAttentionLayer_64587718197528.

Reference computation (B=4, L=1024, D_MODEL=1024, S=2048, T=NUM_TOKENS=1000,
H=16, E=256, D_LLM=4096):
    q = (X @ Wq.T + bq)            [B*L, H*E]      X = target_embedding
    k = (SE @ Wk.T + bk)           [S, H*E]        SE = source_embedding
    v = (VE @ Wv.T + bv)           [S, H*E]        VE = value_embedding
    scores[b,h,l,s] = q . k / 16 ; A = softmax_s ; out = A @ v
    y = out @ Wo.T + bo            [B*L, D_LLM]

Sharding: tensor-parallel over heads. Core i owns heads {2i, 2i+1} (an
e-slice of 512 of the H*E dim). Each core computes its q/k/v projections,
attention for its 2 heads, and a partial out-projection
  partial_i = attn_out_i @ Wo[:, sl_i].T          [B*L, D_LLM]
The host sums the 8 partials and adds bo (linearity of the projection).

All matmul operands are bf16 (1.0 PE cycles/row, same as fp32r, but half
the DMA/SBUF traffic); PSUM accumulation and the output partial are fp32.
Phases (PE is kept continuously busy; rel err ~1e-3 vs 2e-2 gate):
  Qproj: all 8 l-chunks of qT projected up front (xt read ONCE), covering
         the st/vt/weight DMA window that previously starved the PE.
  KV:    kT[512,2048] = Wk_i @ SE.T and v[2048,512] = VE_aug @ Wv_aug
         (bias for v folded via ones-row augmentation), SBUF-resident;
         st/vt staged whole-row [128,2048] for 4KB DMA lines.
  Attn+Proj interleaved per l-chunk: scoresT in PSUM, exp on ACT
         (scale=1/16, no max subtraction -- |scaled scores| < ~8),
         softmax denominators via DVE accumulation + ones-matmul
         partition-reduce, outT normalized by reciprocal broadcast; then
         the out-projection for that l-chunk immediately (spreads the
         67MB fp32 partial write across the whole attention span).
"""
import numpy as np

# ---- problem constants (hardcoded per contract) ----
B, L, D = 4, 1024, 1024
S, T = 2048, 1000
H, E = 16, 256
DL = 4096
BL = B * L            # 4096 query rows
EC = 512              # e-slice per core (2 heads)
NCORES = 8
T1 = T + 1            # augmented contraction for v bias

_CACHE = {}


def _build_nc():
    from contextlib import ExitStack

    import concourse.tile as tile
    from concourse import bacc, mybir

    F32 = mybir.dt.float32
    F32R = mybir.dt.float32r
    BF16 = mybir.dt.bfloat16
    AF = mybir.ActivationFunctionType
    MUL = mybir.AluOpType.mult
    ADD = mybir.AluOpType.add

    nc = bacc.Bacc("TRN2", target_bir_lowering=False, debug=False,
                   num_devices=NCORES)

    xt = nc.dram_tensor("xt", [D, BL], BF16, kind="ExternalInput")
    st = nc.dram_tensor("st", [T, S], BF16, kind="ExternalInput")
    vt = nc.dram_tensor("vt", [T1, S], BF16, kind="ExternalInput")
    wqt = nc.dram_tensor("wqt", [D, EC], BF16, kind="ExternalInput")
    wkt = nc.dram_tensor("wkt", [T, EC], BF16, kind="ExternalInput")
    wvt = nc.dram_tensor("wvt", [T1, EC], BF16, kind="ExternalInput")
    wot = nc.dram_tensor("wot", [EC, DL], BF16, kind="ExternalInput")
    bq_d = nc.dram_tensor("bq", [EC], F32, kind="ExternalInput")
    bk_d = nc.dram_tensor("bk", [EC], F32, kind="ExternalInput")
    out_d = nc.dram_tensor("out", [BL, DL], F32, kind="ExternalOutput")

    NKD = 8            # k-tiles for D=1024
    NKT = 8            # k-tiles for T=1000/T1=1001 (last partial)
    NLC = BL // 512    # 8 l-chunks
    NST = S // 128     # 16 s-tiles

    def kp_of(kk, total):
        return min(128, total - kk * 128)

    with tile.TileContext(nc) as tc:
        with ExitStack() as root:
            root.enter_context(
                nc.allow_low_precision(reason="bf16 matmul pipeline"))

            # ---- persistent pools ----
            consts = root.enter_context(tc.tile_pool(name="consts", bufs=1))
            kvp = root.enter_context(tc.tile_pool(name="kv", bufs=1))
            qtp = root.enter_context(tc.tile_pool(name="qt", bufs=1))
            wqp = root.enter_context(tc.tile_pool(name="wq", bufs=1))
            wop = root.enter_context(tc.tile_pool(name="wo", bufs=1))

            ones_f = consts.tile([128, 128], F32, name="ones_f")
            nc.vector.memset(ones_f[:], 1.0)
            ones_m = consts.tile([128, 128], F32R, name="ones_m")
            nc.vector.tensor_copy(ones_m[:], ones_f[:])
            bqbk_t = consts.tile([128, 8], F32, name="bqbk_t")
            bq_t = bqbk_t[:, 0:4]
            bk_t = bqbk_t[:, 4:8]

            # kT: 4 e-tiles x [128, S]; v: 4 groups [128, 4*EC]
            kt_sb = [kvp.tile([128, S], BF16, name=f"kt{m}", tag=f"kt{m}")
                     for m in range(4)]
            v_sb = [kvp.tile([128, 4 * EC], BF16, name=f"v{g}", tag=f"v{g}")
                    for g in range(4)]
            # qT for ALL l-chunks: 4 e-tiles x [128, BL]
            qt_all = [qtp.tile([128, BL], BF16, name=f"qT{m}", tag=f"qT{m}")
                      for m in range(4)]
            # Wo k-tiles: 4 x [128, DL], fully resident before proj
            wo_sb = [wop.tile([128, DL], BF16, name=f"wo{ke}", tag=f"wo{ke}")
                     for ke in range(4)]

            # ---- phase Qproj (covers weight DMA) + KV ----
            with ExitStack() as ph:
                ph.enter_context(nc.named_scope("qkv"))
                wkv_pool = ph.enter_context(tc.tile_pool(name="wkv", bufs=1))

                wq_t, wk_t, wv_t = [], [], []
                for kk in range(NKD):
                    wq_t.append(wqp.tile([128, EC], BF16, name=f"wq{kk}"))
                for kk in range(NKT):
                    wk_t.append(wkv_pool.tile([128, EC], BF16, name=f"wk{kk}"))
                    wv_t.append(wkv_pool.tile([128, EC], BF16, name=f"wv{kk}"))

                # qproj: for each 512-wide l-chunk, 4 PSUM tiles (one per
                # e-tile), double-buffered generations so evictions overlap
                # the next chunk's matmuls. Evictions rotate over
                # ACT/DVE/Pool. First DMAs are interleaved wq/xq so the
                # first matmul's operands land as early as possible.
                with ExitStack() as qph:
                    xq_pool = qph.enter_context(
                        tc.tile_pool(name="xq", bufs=20))
                    psq = qph.enter_context(
                        tc.tile_pool(name="psq", bufs=2, space="PSUM"))
                    for lcq in range(NLC):
                        ps_q = [psq.tile([128, 512], F32, tag=f"psq{m}",
                                         name=f"psq{m}") for m in range(4)]
                        for kk in range(NKD):
                            if lcq == 0:
                                # first weight tile races the first xq on a
                                # separate queue; the rest ride scalar
                                weng = nc.sync if kk == 0 else nc.scalar
                                weng.dma_start(
                                    wq_t[kk][:],
                                    wqt[kk * 128:(kk + 1) * 128, :])
                            xq = xq_pool.tile([128, 512], BF16, tag="xq",
                                              name="xq")
                            # SP queue carries only the xq stream (in-order
                            # queue: anything else ahead of an xq tile
                            # delays it and starves the PE)
                            nc.sync.dma_start(
                                xq[:], xt[kk * 128:(kk + 1) * 128,
                                          lcq * 512:(lcq + 1) * 512])
                            if lcq == 0 and kk == 2:
                                # biases off the critical first-DMA path but
                                # in time for the first eviction
                                nc.scalar.dma_start(
                                    bqbk_t[:, 0:4],
                                    bq_d.ap().rearrange("(m p) -> p m",
                                                        p=128))
                                nc.scalar.dma_start(
                                    bqbk_t[:, 4:8],
                                    bk_d.ap().rearrange("(m p) -> p m",
                                                        p=128))
                            for m in range(4):
                                nc.tensor.matmul(
                                    ps_q[m][:],
                                    wq_t[kk][:, m * 128:(m + 1) * 128],
                                    xq[:], start=(kk == 0),
                                    stop=(kk == NKD - 1))
                        for m in range(4):
                            dst = qt_all[m][:, lcq * 512:(lcq + 1) * 512]
                            if m == 0:
                                nc.scalar.activation(
                                    dst, ps_q[m][:], AF.Identity,
                                    bias=bq_t[:, m:m + 1])
                            elif m == 1:
                                nc.vector.tensor_scalar_add(
                                    dst, ps_q[m][:], bq_t[:, m:m + 1])
                            elif m == 2:
                                nc.vector.tensor_scalar_add(
                                    dst, ps_q[m][:], bq_t[:, m:m + 1])
                            else:
                                nc.scalar.activation(
                                    dst, ps_q[m][:], AF.Identity,
                                    bias=bq_t[:, m:m + 1])
                        # stream wk/wv weight tiles on the scalar queue
                        # (keeps the SP queue clear for xq)
                        if lcq % 2 == 1:
                            kk = lcq // 2
                            kp = kp_of(kk, T)
                            kp1 = kp_of(kk, T1)
                            nc.scalar.dma_start(
                                wk_t[kk][:kp, :],
                                wkt[kk * 128:kk * 128 + kp, :])
                            nc.scalar.dma_start(
                                wv_t[kk][:kp1, :],
                                wvt[kk * 128:kk * 128 + kp1, :])
                    for kk in range(4, NKT):
                        kp = kp_of(kk, T)
                        kp1 = kp_of(kk, T1)
                        nc.scalar.dma_start(
                            wk_t[kk][:kp, :], wkt[kk * 128:kk * 128 + kp, :])
                        nc.scalar.dma_start(
                            wv_t[kk][:kp1, :],
                            wvt[kk * 128:kk * 128 + kp1, :])

                # ---- KV: kT = Wk_i @ SE.T ; v = VE_aug @ Wv_aug ----
                # st/vt stream per (sc, kk); wo preload spread across sc.
                sk_pool = ph.enter_context(tc.tile_pool(name="sk", bufs=12))
                sv_pool = ph.enter_context(tc.tile_pool(name="sv", bufs=12))
                psk = ph.enter_context(
                    tc.tile_pool(name="psk", bufs=1, space="PSUM"))
                psv = ph.enter_context(
                    tc.tile_pool(name="psv", bufs=1, space="PSUM"))
                for sc in range(S // 512):
                    nc.scalar.dma_start(
                        wo_sb[sc][:], wot[sc * 128:(sc + 1) * 128, :])
                    ps_k = [psk.tile([128, 512], F32, tag=f"psk{m}",
                                     name=f"psk{m}") for m in range(4)]
                    for kk in range(NKT):
                        kp = kp_of(kk, T)
                        stt = sk_pool.tile([128, 512], BF16, tag="stt",
                                           name="stt")
                        nc.sync.dma_start(
                            stt[:kp, :], st[kk * 128:kk * 128 + kp,
                                            sc * 512:(sc + 1) * 512])
                        for m in range(4):
                            nc.tensor.matmul(
                                ps_k[m][:],
                                wk_t[kk][:kp, m * 128:(m + 1) * 128],
                                stt[:kp, :],
                                start=(kk == 0), stop=(kk == NKT - 1))
                    for m in range(4):
                        nc.scalar.activation(
                            kt_sb[m][:, sc * 512:(sc + 1) * 512], ps_k[m][:],
                            AF.Identity, bias=bk_t[:, m:m + 1])
                    ps_v = [psv.tile([128, 512], F32, tag=f"psv{j}",
                                     name=f"psv{j}") for j in range(4)]
                    for kk in range(NKT):
                        kp1 = kp_of(kk, T1)
                        vtt = sv_pool.tile([128, 512], BF16, tag="vtt",
                                           name="vtt")
                        nc.sync.dma_start(
                            vtt[:kp1, :], vt[kk * 128:kk * 128 + kp1,
                                             sc * 512:(sc + 1) * 512])
                        for j in range(4):
                            nc.tensor.matmul(
                                ps_v[j][:],
                                vtt[:kp1, j * 128:(j + 1) * 128],
                                wv_t[kk][:kp1, :],
                                start=(kk == 0), stop=(kk == NKT - 1))
                    for j in range(4):
                        nc.scalar.activation(
                            v_sb[sc][:, j * EC:(j + 1) * EC], ps_v[j][:],
                            AF.Copy)

            # ---- fused attention + out-projection per l-chunk ----
            with ExitStack() as ph:
                ph.enter_context(nc.named_scope("attnproj"))
                a_pool = ph.enter_context(tc.tile_pool(name="ap", bufs=1))
                acc_pool = ph.enter_context(tc.tile_pool(name="accp", bufs=2))
                bc_pool = ph.enter_context(tc.tile_pool(name="bcp", bufs=2))
                o_pool = ph.enter_context(tc.tile_pool(name="op", bufs=2))
                pev_pool = ph.enter_context(tc.tile_pool(name="pev", bufs=6))
                ps_sT_p = ph.enter_context(
                    tc.tile_pool(name="ps_sT", bufs=2, space="PSUM"))
                ps_o_p = ph.enter_context(
                    tc.tile_pool(name="ps_o", bufs=1, space="PSUM"))
                psp_p = ph.enter_context(
                    tc.tile_pool(name="psp", bufs=2, space="PSUM"))

                # Deferred softmax-denominator finish for the previous head:
                # the ones-matmul (partition reduce + broadcast), reciprocal
                # and o-normalization are emitted only after the NEXT head's
                # first scores, so the PE never waits on the DVE chain.
                pend = {"v": None}

                def flush_den():
                    p = pend["v"]
                    if p is None:
                        return
                    acc_d, ps_os_p, o0, o1 = p
                    ps_b = psp_p.tile([128, 1024], F32, tag="pp",
                                      name="pp")[:, 0:512]
                    nc.tensor.matmul(ps_b, ones_m[:], acc_d[:],
                                     start=True, stop=True)
                    bc = bc_pool.tile([128, 512], F32, tag="bc", name="bc")
                    nc.vector.reciprocal_approx_fast(out=bc[:], in_=ps_b)
                    nc.vector.tensor_tensor(o0[:], ps_os_p[0][:], bc[:], MUL)
                    nc.vector.tensor_tensor(o1[:], ps_os_p[1][:], bc[:], MUL)
                    pend["v"] = None

                def attn_head(lc, h, a_t, o_t):
                    # scores per 128-s-tile in alternating 1-bank PSUM tiles
                    # (bufs=2) so the next scores never wait on the exp
                    # draining the previous tile. AV matmuls lag 4 s-tiles
                    # behind; exps/accumulation have a ~3us window.
                    acc_d = acc_pool.tile([128, 512], F32R, tag="accd",
                                          name="accd")
                    acc_p = acc_pool.tile([128, 512], F32R, tag="accp",
                                          name="accp")
                    ps_os = [ps_o_p.tile([128, 512], F32, tag=f"os{et}",
                                         name=f"os{et}") for et in range(2)]

                    def av_stt(stt):
                        for et in range(2):
                            nc.tensor.matmul(
                                ps_os[et][:],
                                v_sb[stt // 4][:, (stt % 4) * EC + h * E
                                               + et * 128:
                                               (stt % 4) * EC + h * E
                                               + (et + 1) * 128],
                                a_t[stt // 4][:, (stt % 4) * 512:
                                              (stt % 4 + 1) * 512],
                                start=(stt == 0), stop=(stt == NST - 1))

                    AVLAG = 4
                    for stt in range(NST):
                        ps = ps_sT_p.tile([128, 512], F32, tag="sT",
                                          name="sT")
                        for et in range(2):
                            m = 2 * h + et
                            nc.tensor.matmul(
                                ps[:],
                                kt_sb[m][:, stt * 128:(stt + 1) * 128],
                                qt_all[m][:, lc * 512:(lc + 1) * 512],
                                start=(et == 0), stop=(et == 1))
                        if stt == 0:
                            flush_den()
                        a_ap = a_t[stt // 4][:, (stt % 4) * 512:
                                             (stt % 4 + 1) * 512]
                        nc.scalar.activation(a_ap, ps[:], AF.Exp,
                                             scale=0.0625)
                        # denominator partial sums: DVE (even) / Pool (odd)
                        if stt % 2 == 0:
                            if stt == 0:
                                nc.vector.tensor_copy(acc_d[:], a_ap)
                            else:
                                nc.vector.tensor_tensor(acc_d[:], acc_d[:],
                                                        a_ap, ADD)
                        else:
                            if stt == 1:
                                nc.gpsimd.tensor_copy(acc_p[:], a_ap)
                            else:
                                nc.gpsimd.tensor_tensor(acc_p[:], acc_p[:],
                                                        a_ap, ADD)
                        if stt >= AVLAG:
                            av_stt(stt - AVLAG)
                    for stt in range(NST - AVLAG, NST):
                        av_stt(stt)
                    nc.vector.tensor_tensor(acc_d[:], acc_d[:], acc_p[:],
                                            ADD)
                    pend["v"] = (acc_d, ps_os, o_t[2 * h], o_t[2 * h + 1])

                def proj_lc(lc, o_t):
                    # partial[lt,dc] = sum_ke o_t[ke].T @ wo ; dch pairs share
                    # the ke stationary tile (halved LDWEIGHTS traffic).
                    # Groups run in pairs with the ke contraction split in
                    # halves (ke 0-1 of both groups first): h1's o tiles are
                    # only needed ~1.9us into the pair, hiding the normalize
                    # latency of the head just flushed.
                    flush_den()
                    for lt in range(4):
                        for dcp in range(4):
                            pp = psp_p.tile([128, 1024], F32, tag="pp",
                                            name="pp")
                            for ke in range(4):
                                for dch in range(2):
                                    dc = 2 * dcp + dch
                                    nc.tensor.matmul(
                                        pp[:, dch * 512:(dch + 1) * 512],
                                        o_t[ke][:, lt * 128:(lt + 1) * 128],
                                        wo_sb[ke][:, dc * 512:(dc + 1) * 512],
                                        start=(ke == 0), stop=(ke == 3))
                            ev = pev_pool.tile([128, 1024], F32, tag="pev",
                                               name="pev")
                            nc.vector.tensor_copy(ev[:, 0:512], pp[:, 0:512])
                            nc.scalar.activation(ev[:, 512:1024],
                                                 pp[:, 512:1024], AF.Copy)
                            nc.sync.dma_start(
                                out_d[(lc * 4 + lt) * 128:
                                      (lc * 4 + lt + 1) * 128,
                                      dcp * 1024:(dcp + 1) * 1024], ev[:])

                for lc in range(NLC):
                    a_t = [a_pool.tile([128, 4 * 512], BF16, tag=f"a{g}",
                                       name=f"a{g}") for g in range(4)]
                    o_t = [o_pool.tile([128, 512], BF16, tag=f"o{m}",
                                       name=f"o{m}") for m in range(4)]
                    attn_head(lc, 0, a_t, o_t)
                    attn_head(lc, 1, a_t, o_t)
                    proj_lc(lc, o_t)

    nc.compile()
    return nc


def _get_nc():
    if "nc" not in _CACHE:
        _CACHE["nc"] = _build_nc()
    return _CACHE["nc"]


def _build_in_maps(inputs):
    return _prep(**{k: inputs[k] for k in (
        "target_embedding", "source_embedding", "value_embedding",
        "Wq", "bq", "Wk", "bk", "Wv", "bv", "Wo")})


def _prep(target_embedding, source_embedding, value_embedding,
          Wq, bq, Wk, bk, Wv, bv, Wo):
    import ml_dtypes
    mmd = ml_dtypes.bfloat16
    f32 = np.float32
    X = np.asarray(target_embedding, f32).reshape(BL, D)
    xt = np.ascontiguousarray(X.T)                       # [D, BL]
    st = np.ascontiguousarray(np.asarray(source_embedding, f32).T)  # [T, S]
    vt_base = np.asarray(value_embedding, f32).T         # [T, S]
    vt = np.ascontiguousarray(
        np.concatenate([vt_base, np.ones((1, S), f32)], axis=0))  # [T1, S]
    WqT = np.asarray(Wq, f32).T                          # [D, H*E]
    WkT = np.asarray(Wk, f32).T                          # [T, H*E]
    WvT = np.asarray(Wv, f32).T                          # [T, H*E]
    WoT = np.asarray(Wo, f32).T                          # [H*E, DL]
    bq = np.asarray(bq, f32)
    bk = np.asarray(bk, f32)
    bv = np.asarray(bv, f32)

    xt_c = xt.astype(mmd)
    st_c = st.astype(mmd)
    vt_c = vt.astype(mmd)
    in_maps = []
    for i in range(NCORES):
        sl = slice(i * EC, (i + 1) * EC)
        wvt_i = np.ascontiguousarray(
            np.concatenate([WvT[:, sl], bv[sl][None, :]], axis=0))  # [T1, EC]
        in_maps.append({
            "xt": xt_c,
            "st": st_c,
            "vt": vt_c,
            "wqt": np.ascontiguousarray(WqT[:, sl]).astype(mmd),
            "wkt": np.ascontiguousarray(WkT[:, sl]).astype(mmd),
            "wvt": wvt_i.astype(mmd),
            "wot": np.ascontiguousarray(WoT[sl, :]).astype(mmd),
            "bq": np.ascontiguousarray(bq[sl]),
            "bk": np.ascontiguousarray(bk[sl]),
        })
    return in_maps


def kernel(target_embedding, source_embedding, value_embedding,
           Wq, bq, Wk, bk, Wv, bv, Wo, bo):
    from concourse.bass_utils import run_bass_kernel_spmd

    in_maps = _prep(target_embedding, source_embedding, value_embedding,
                    Wq, bq, Wk, bk, Wv, bv, Wo)
    _CACHE["in_maps"] = in_maps
    nc = _get_nc()
    res = run_bass_kernel_spmd(nc, in_maps, list(range(NCORES)))

    acc = res.results[0]["out"].astype(np.float64)
    for i in range(1, NCORES):
        acc += res.results[i]["out"]
    out = (acc + np.asarray(bo, np.float64)[None, :]).astype(np.float32)
    return out.reshape(B, L, DL)
